# revision 1
# baseline (speedup 1.0000x reference)
"""AGNN (AMNet) message-passing kernel for 8 TRN2 NeuronCores.

Strategy (graph/data parallel per the sharding hint):
  - Nodes are partitioned contiguously across the 8 cores by dst.
  - Each core runs the input MLP for its 12.5k nodes, L2-normalizes, packs
    rows [hn(64) | h(64)] as bf16 (256B), appends a zero pad row, and one
    AllGather replicates the [100008, 128] table to every core's HBM
    (random edges make the halo the whole graph).
  - Host pre-buckets each core's incoming edges into a dense degree-sorted
    per-dst slot layout: blocks of 128 dsts x K slots.  dma_gather's int16
    index limit forces 4 table chunks (2 cores each, 25002 rows); per
    (block, chunk) the slot count is the block max, pad slots point at the
    chunk's zero row (contributing exp(0)=1, subtracted exactly via a
    host-computed pad count).  Softmax needs no max-subtraction since
    |beta * cos| <= |beta|.
  - Per super-block of blocks: 4 dma_gathers (one per chunk) fill the slot
    tile; dots vs local dst rows (DVE), exp+sum (ACT), weighted k-sum via
    PE identity-matmul accumulation, then the classifier.

kernel() accepts FULL inputs and returns the FULL [N, 2] float32 output.
"""

import math
import sys

sys.path.insert(0, "/opt/trn_rl_repo")

import numpy as np

CORES = 8
P = 128
NCH = 4  # table chunks (int16 index reach)
SBB = 2  # blocks per super-block (per dma_gather group)


# ----------------------------------------------------------------- host prep
def _wrap_idx(flat):
    """dma_gather index layout: wrapped in 16 partitions, replicated x8."""
    n = flat.shape[0]
    w = flat.reshape(n // 16, 16).T
    return np.tile(w, (8, 1)).astype(np.int16)


def _preprocess(x, edge_index, n_cores=CORES):
    N, IN = x.shape
    NP = N // n_cores
    assert NP * n_cores == N
    NB = math.ceil(NP / P)
    NBP = NB * P
    STRIDE = NP + 1  # per-core table rows incl zero row
    CHROWS = (n_cores // NCH) * STRIDE

    src = np.asarray(edge_index[0], dtype=np.int64)
    dst = np.asarray(edge_index[1], dtype=np.int64)
    chunk_of_src = src // ((n_cores // NCH) * NP)  # = core(src) // 2

    # CSR by (dst, chunk)
    order = np.lexsort((chunk_of_src, dst))
    src_sorted = src[order]
    counts = np.bincount(dst, minlength=N).astype(np.int64)
    row_start = np.zeros(N + 1, np.int64)
    np.cumsum(counts, out=row_start[1:])

    perm = np.zeros((n_cores, NP), np.int64)
    Kpc = np.zeros((n_cores, NB, NCH), np.int64)
    cnt_all = []
    for p in range(n_cores):
        lo, hi = p * NP, (p + 1) * NP
        m = (dst >= lo) & (dst < hi)
        cnt = np.zeros((NP, NCH), np.int64)
        np.add.at(cnt, (dst[m] - lo, chunk_of_src[m]), 1)
        own_chunk = p // (n_cores // NCH)
        cnt[:, own_chunk] += 1  # self loop
        cnt_all.append(cnt)
        ol = np.lexsort((-cnt[:, 3], -cnt[:, 2], -cnt[:, 1], -cnt[:, 0]))
        perm[p] = lo + ol
        cs = cnt[ol]
        for i in range(NB):
            Kpc[p, i] = cs[i * P : (i + 1) * P].max(axis=0)
    K = Kpc.max(axis=0)  # [NB, NCH] common schedule
    Ktot = K.sum(axis=1)  # [NB]

    # permuted-table local (within-chunk) row index of each node
    pos = np.zeros(N, np.int64)
    for p in range(n_cores):
        pos[perm[p]] = np.arange(NP)
    core_of = np.arange(N) // NP
    localidx = (core_of % (n_cores // NCH)) * STRIDE + pos  # within chunk
    PADLOC = NP  # zero row of the chunk's first core

    # super-block grouping
    sbs = [list(range(s, min(s + SBB, NB))) for s in range(0, NB, SBB)]

    # per-core index streams (wrapped), common column layout
    # column layout: for each sb, for each chunk c: 8 * sum_b K[b][c] columns
    sb_meta = []  # per sb: dict(cols_off, per-chunk (ncols, nidx), block views)
    coloff = 0
    for sb in sbs:
        chinfo = []
        soff = 0
        for c in range(NCH):
            nidx = int(P * sum(K[b][c] for b in sb))
            ncols = nidx // 16
            chinfo.append(dict(c=c, nidx=nidx, ncols=ncols, coloff=coloff,
                               soff=soff))
            coloff += ncols
            soff += nidx // P
        sb_meta.append(dict(blocks=sb, chunks=chinfo, S=soff))
    TOTCOLS = coloff

    idx_all = np.zeros((n_cores, P, TOTCOLS), np.int16)
    negnpad = np.zeros((n_cores, NBP), np.float32)
    for p in range(n_cores):
        cnt = cnt_all[p]
        own = p // (n_cores // NCH)
        cnt_e = cnt.copy()
        cnt_e[:, own] -= 1  # edge-only counts (self loop is not in the CSR)
        permp = perm[p]
        for meta in sb_meta:
            for ch in meta["chunks"]:
                c = ch["c"]
                parts = []
                for b in meta["blocks"]:
                    Kbc = int(K[b][c])
                    arr = np.full((Kbc, P), PADLOC, np.int64)
                    base = b * P
                    hi = min(P, NP - base)
                    for d in range(hi):
                        g = permp[base + d]
                        gl = g - p * NP
                        s0 = row_start[g] + cnt_e[gl, :c].sum()
                        nbc = cnt_e[gl, c]
                        arr[:nbc, d] = localidx[src_sorted[s0 : s0 + nbc]]
                        if c == own:
                            arr[nbc, d] = localidx[g]
                    parts.append(arr.ravel())
                if parts:
                    flat = np.concatenate(parts)
                    if flat.shape[0]:
                        idx_all[p][:, ch["coloff"] : ch["coloff"] + ch["ncols"]] = (
                            _wrap_idx(flat)
                        )
        deg = cnt.sum(axis=1)[permp - p * NP]
        for b in range(NB):
            base = b * P
            hi = min(P, NP - base)
            negnpad[p, base : base + hi] = -(float(Ktot[b]) - deg[base : base + hi])
            negnpad[p, base + hi : base + P] = -(float(Ktot[b]) - 1.0)

    return dict(
        N=N, IN=IN, NP=NP, NB=NB, NBP=NBP, STRIDE=STRIDE, CHROWS=CHROWS,
        K=K, Ktot=Ktot, sb_meta=sb_meta, TOTCOLS=TOTCOLS,
        perm=perm, idx_all=idx_all, negnpad=negnpad,
    )


# ------------------------------------------------------------------ builder
def _patch_walrus_args():
    """bass_utils' walrus invocation omits --dge-levels (DynamicDMA off);
    inject the standard neuronxcc gen3 set."""
    import concourse.bass_utils as bu

    if getattr(bu, "_agnn_dge_patch", False):
        return
    orig = bu.get_walrus_args

    def patched(*a, **k):
        return list(orig(*a, **k)) + [
            "--dge-levels=io,spill_reload,scalar_dynamic_offset,"
            "vector_dynamic_offsets,dst_reduce,transpose",
        ]

    bu.get_walrus_args = patched
    bu._agnn_dge_patch = True


def _split_multi_waits(nc):
    """This walrus caps sync waits at 1/instruction (2 for EventSemaphore);
    split extra waits onto preceding same-engine NOPs."""
    import bass_rust
    import concourse.mybir as mybir

    cnt = 0
    for func in nc.m.functions:
        for block in func.blocks:
            out = []
            for inst in block.instructions:
                si = inst.sync_info
                cap = 2 if isinstance(inst, mybir.InstEventSemaphore) else 1
                if (
                    si is not None
                    and si.on_wait
                    and len(si.on_wait) > cap
                    and inst.engine is not None
                ):
                    waits = list(si.on_wait)
                    for w in waits[cap:]:
                        cnt += 1
                        nop = mybir.InstNoOp(
                            name=f"wsplit{cnt}", engine=inst.engine, ins=[], outs=[]
                        )
                        nop.sync_info = bass_rust.SyncInfo(on_wait=[w], on_update=[])
                        try:
                            nc.register_instruction(nop, overwrite=True)
                        except Exception:
                            pass
                        out.append(nop)
                    inst.sync_info = bass_rust.SyncInfo(
                        on_wait=waits[:cap], on_update=list(si.on_update or [])
                    )
                out.append(inst)
            try:
                block.instructions = out
            except Exception:
                block.instructions.clear()
                block.instructions.extend(out)
    return cnt


def _build_nc(cfg):
    _patch_walrus_args()
    import concourse.bacc as bacc
    import concourse.mybir as mybir
    import concourse.tile as tile
    from concourse.tile import add_dep_helper
    from concourse.masks import make_identity

    N, IN = cfg["N"], cfg["IN"]
    HID, C = cfg["HID"], cfg["C"]
    NP, NB, NBP = cfg["NP"], cfg["NB"], cfg["NBP"]
    STRIDE, CHROWS = cfg["STRIDE"], cfg["CHROWS"]
    K, Ktot = cfg["K"], cfg["Ktot"]
    sb_meta = cfg["sb_meta"]
    TOTCOLS = cfg["TOTCOLS"]
    n_cores = cfg["n_cores"]
    D2 = 2 * HID
    TROWS = n_cores * STRIDE

    f32 = mybir.dt.float32
    bf16 = mybir.dt.bfloat16

    nc = bacc.Bacc("TRN2", num_devices=n_cores, dynamic_dma_scratch_size=65536)

    xT = nc.declare_dram_parameter("xT", [IN, NBP], f32, isOutput=False)
    idx_t = nc.declare_dram_parameter("idx", [P, TOTCOLS], mybir.dt.int16, isOutput=False)
    nnp_t = nc.declare_dram_parameter("negnpad", [NBP], f32, isOutput=False)
    W1_t = nc.declare_dram_parameter("W1", [IN, HID], f32, isOutput=False)
    b1_t = nc.declare_dram_parameter("b1", [HID, 1], f32, isOutput=False)
    W2_t = nc.declare_dram_parameter("W2", [HID, HID], f32, isOutput=False)
    b2_t = nc.declare_dram_parameter("b2", [HID, 1], f32, isOutput=False)
    Wc_t = nc.declare_dram_parameter("Wc", [HID, C], f32, isOutput=False)
    bc_t = nc.declare_dram_parameter("bc", [C, 1], f32, isOutput=False)
    beta_t = nc.declare_dram_parameter("beta128", [P, 1], f32, isOutput=False)
    out_t = nc.declare_dram_parameter("out", [C, NBP], f32, isOutput=True)

    with tile.TileContext(nc) as tc:
        gsem = nc.alloc_semaphore("gsem")
        agin, free_agin = tc.tile([STRIDE, D2], bf16, space="DRAM", name="agin")
        table, free_table = tc.tile(
            [TROWS, D2], bf16, space="DRAM", name="table"
        )

        with tc.tile_pool(name="consts", bufs=1) as cpool:
            W1sb = cpool.tile([IN, HID], f32)
            W2sb = cpool.tile([HID, HID], f32)
            Wcsb = cpool.tile([HID, C], f32)
            b1sb = cpool.tile([HID, 1], f32)
            b2sb = cpool.tile([HID, 1], f32)
            bcsb = cpool.tile([C, 1], f32)
            betasb = cpool.tile([P, 1], f32)
            ident = cpool.tile([P, P], f32)
            identb = cpool.tile([P, P], bf16)
            hnD = cpool.tile([P, NB, HID], bf16)
            yT = cpool.tile([C, NBP], f32)
            zrow = cpool.tile([1, D2], bf16)

            nc.sync.dma_start(out=W1sb[:], in_=W1_t[:])
            nc.sync.dma_start(out=W2sb[:], in_=W2_t[:])
            nc.sync.dma_start(out=Wcsb[:], in_=Wc_t[:])
            nc.sync.dma_start(out=b1sb[:], in_=b1_t[:])
            nc.sync.dma_start(out=b2sb[:], in_=b2_t[:])
            nc.sync.dma_start(out=bcsb[:], in_=bc_t[:])
            nc.sync.dma_start(out=betasb[:], in_=beta_t[:])
            make_identity(nc, ident[:])
            nc.vector.tensor_copy(identb[:], ident[:])
            nc.gpsimd.memset(zrow[:], 0)
            nc.sync.dma_start(out=agin[NP : NP + 1, :], in_=zrow[:])

            # ---------------- phase 1: MLP + packed table build ------------
            with (
                tc.tile_pool(name="mlp", bufs=3) as mpool,
                tc.tile_pool(name="mlpp", bufs=2, space="PSUM") as mpp,
            ):
                for i in range(NB):
                    lo = i * P
                    hi = min(NP, lo + P)
                    xt = mpool.tile([IN, P], f32, tag="xt")
                    nc.sync.dma_start(out=xt[:], in_=xT[:, lo : lo + P])
                    ps1 = mpp.tile([HID, P], f32, tag="ps1")
                    nc.tensor.matmul(ps1[:], lhsT=W1sb[:], rhs=xt[:], start=True, stop=True)
                    r1 = mpool.tile([HID, P], f32, tag="r1")
                    nc.scalar.activation(
                        r1[:], ps1[:], mybir.ActivationFunctionType.Relu, bias=b1sb[:]
                    )
                    ps2 = mpp.tile([HID, P], f32, tag="ps2")
                    nc.tensor.matmul(ps2[:], lhsT=W2sb[:], rhs=r1[:], start=True, stop=True)
                    hT = mpool.tile([HID, P], f32, tag="hT")
                    nc.scalar.activation(
                        hT[:], ps2[:], mybir.ActivationFunctionType.Identity, bias=b2sb[:]
                    )
                    pst = mpp.tile([P, HID], f32, tag="pst")
                    nc.tensor.transpose(pst[:], hT[:], ident[0:HID, 0:HID])
                    h = mpool.tile([P, HID], f32, tag="h")
                    nc.scalar.activation(h[:], pst[:], mybir.ActivationFunctionType.Copy)
                    sq = mpool.tile([P, HID], f32, tag="sq")
                    ss = mpool.tile([P, 1], f32, tag="ss")
                    nc.vector.scalar_tensor_tensor(
                        out=sq[:], in0=h[:], scalar=0.0, in1=h[:],
                        op0=mybir.AluOpType.bypass, op1=mybir.AluOpType.mult,
                        accum_out=ss[:],
                    )
                    nrm = mpool.tile([P, 1], f32, tag="nrm")
                    nc.scalar.activation(nrm[:], ss[:], mybir.ActivationFunctionType.Sqrt)
                    nc.vector.tensor_scalar_max(out=nrm[:], in0=nrm[:], scalar1=1e-12)
                    rnrm = mpool.tile([P, 1], f32, tag="rnrm")
                    nc.vector.reciprocal(rnrm[:], nrm[:])
                    tab = mpool.tile([P, D2], bf16, tag="tab")
                    nc.scalar.activation(
                        tab[:, 0:HID], h[:], mybir.ActivationFunctionType.Copy,
                        scale=rnrm[:],
                    )
                    nc.vector.tensor_copy(tab[:, HID:D2], h[:])
                    nc.vector.tensor_copy(hnD[:, i, :], tab[:, 0:HID])
                    nc.sync.dma_start(out=agin[lo:hi, :], in_=tab[0 : hi - lo, :])

            # ---------------- phase 2: AllGather the table -----------------
            nc.gpsimd.collective_compute(
                "AllGather",
                mybir.AluOpType.bypass,
                replica_groups=[list(range(n_cores))],
                ins=[agin[:].opt()],
                outs=[table[:].opt()],
            )

            # ---------------- phase 3: per-super-block AGNN ----------------
            gcount = 0
            last_wts = None
            pending = None  # (consumer insts, ...) of previous sb
            with (
                tc.tile_pool(name="blk", bufs=2) as bpool,
                tc.tile_pool(name="blks", bufs=2) as spool,
                tc.tile_pool(name="blkp", bufs=2, space="PSUM") as bpp,
            ):
                for meta in sb_meta:
                    blocks = meta["blocks"]
                    S = meta["S"]
                    ncols_sb = sum(ch["ncols"] for ch in meta["chunks"])
                    coloff0 = meta["chunks"][0]["coloff"]
                    idxsb = bpool.tile([P, ncols_sb], mybir.dt.int16, tag="idx")
                    nc.sync.dma_start(
                        out=idxsb[:], in_=idx_t[:, coloff0 : coloff0 + ncols_sb]
                    )
                    slots = bpool.tile([P, S, D2], bf16, tag="slots")
                    import os as _os
                    if _os.environ.get("AGNN_ABLATE"):
                        nc.vector.memset(slots[:], 0)
                        class _GW:  # placeholder
                            ins = None
                        gw = _GW()
                        sb_consumers = []
                        _skip_gather = True
                    else:
                        _skip_gather = False
                    with tc.tile_critical():
                        if _skip_gather:
                            nc.sync.nop()
                        for ch in (() if _skip_gather else meta["chunks"]):
                            c, nidx = ch["c"], ch["nidx"]
                            if nidx == 0:
                                continue
                            nc.gpsimd.dma_gather(
                                slots[:, ch["soff"] : ch["soff"] + nidx // P, :],
                                table[c * CHROWS : (c + 1) * CHROWS, :],
                                idxsb[:, ch["coloff"] - coloff0 :
                                      ch["coloff"] - coloff0 + ch["ncols"]],
                                nidx, nidx, D2,
                                single_packet=False,
                            ).then_inc(gsem, 16)
                            gcount += 1
                        if not _skip_gather:
                            gw = nc.gpsimd.wait_ge(gsem, 16 * gcount)
                    sb_consumers = []

                    for bi, b in enumerate(blocks):
                        Ktot_b = int(Ktot[b])
                        lo = b * P
                        nn = bpool.tile([P, 1], f32, tag="nn")
                        nc.sync.dma_start(out=nn[:], in_=nnp_t[lo : lo + P, None])
                        delta = spool.tile([P, Ktot_b], f32, tag="delta")
                        dcoff = 0
                        boffs = []
                        for ch in meta["chunks"]:
                            c = ch["c"]
                            Kbc = int(K[b][c])
                            if Kbc == 0:
                                boffs.append(None)
                                continue
                            so = ch["soff"] + sum(int(K[b2][c]) for b2 in blocks[:bi])
                            boffs.append((so, dcoff, Kbc))
                            tmp = spool.tile([P, Kbc, HID], bf16, tag="tmp")
                            ins0 = nc.vector.tensor_tensor(
                                out=tmp[:],
                                in0=slots[:, so : so + Kbc, 0:HID],
                                in1=hnD[:, b, :].unsqueeze(1).to_broadcast([P, Kbc, HID]),
                                op=mybir.AluOpType.mult,
                            )
                            sb_consumers.append(ins0.ins)
                            nc.vector.tensor_reduce(
                                out=delta[:, dcoff : dcoff + Kbc],
                                in_=tmp[:], axis=mybir.AxisListType.X,
                                op=mybir.AluOpType.add,
                            )
                            dcoff += Kbc
                        ex = spool.tile([P, Ktot_b], bf16, tag="ex")
                        s_raw = bpool.tile([P, 1], f32, tag="sraw")
                        nc.scalar.activation(
                            ex[:], delta[:], mybir.ActivationFunctionType.Exp,
                            scale=betasb[:], accum_out=s_raw[:],
                        )
                        s = bpool.tile([P, 1], f32, tag="s")
                        nc.vector.tensor_tensor(
                            out=s[:], in0=s_raw[:], in1=nn[:], op=mybir.AluOpType.add
                        )
                        rcp = bpool.tile([P, 1], f32, tag="rcp")
                        nc.vector.reciprocal(rcp[:], s[:])

                        pso = bpp.tile([P, HID], f32, tag="pso")
                        nmm = sum(1 for bo in boffs if bo) and Ktot_b
                        mm_i = 0
                        for bo in boffs:
                            if bo is None:
                                continue
                            so, dcoff_c, Kbc = bo
                            wts = spool.tile([P, Kbc, HID], bf16, tag="wts")
                            wi = nc.vector.tensor_tensor(
                                out=wts[:],
                                in0=slots[:, so : so + Kbc, HID:D2],
                                in1=ex[:, dcoff_c : dcoff_c + Kbc]
                                .unsqueeze(2).to_broadcast([P, Kbc, HID]),
                                op=mybir.AluOpType.mult,
                            )
                            sb_consumers.append(wi.ins)
                            last_wts = wi.ins
                            for k in range(Kbc):
                                nc.tensor.matmul(
                                    pso[:], lhsT=identb[:], rhs=wts[:, k, :],
                                    start=(mm_i == 0), stop=(mm_i == nmm - 1),
                                )
                                mm_i += 1
                        scaled = bpool.tile([P, HID], f32, tag="scaled")
                        nc.scalar.activation(
                            scaled[:], pso[:], mybir.ActivationFunctionType.Copy,
                            scale=rcp[:],
                        )
                        pst2 = bpp.tile([HID, P], f32, tag="pst2")
                        nc.tensor.transpose(pst2[:], scaled[:], ident[:])
                        outTs = bpool.tile([HID, P], f32, tag="outTs")
                        nc.scalar.activation(
                            outTs[:], pst2[:], mybir.ActivationFunctionType.Copy
                        )
                        psy = bpp.tile([C, P], f32, tag="psy")
                        nc.tensor.matmul(psy[:], lhsT=Wcsb[:], rhs=outTs[:], start=True, stop=True)
                        nc.scalar.activation(
                            yT[:, lo : lo + P], psy[:],
                            mybir.ActivationFunctionType.Identity, bias=bcsb[:],
                        )

            nc.sync.dma_start(out=out_t[:], in_=yT[:])

        free_agin()
        free_table()

    nc.compile()
    _split_multi_waits(nc)
    return nc


# ------------------------------------------------------------------- driver
_CACHE = {}


def _get_nc(cfg_key, cfg):
    if cfg_key not in _CACHE:
        _CACHE[cfg_key] = _build_nc(cfg)
    return _CACHE[cfg_key]


def _make_in_maps(inputs, prep, n_cores=CORES):
    x = np.asarray(inputs["x"], dtype=np.float32)
    W1 = np.asarray(inputs["W1"], dtype=np.float32)
    b1 = np.asarray(inputs["b1"], dtype=np.float32).reshape(-1, 1)
    W2 = np.asarray(inputs["W2"], dtype=np.float32)
    b2 = np.asarray(inputs["b2"], dtype=np.float32).reshape(-1, 1)
    Wc = np.asarray(inputs["Wc"], dtype=np.float32)
    bc = np.asarray(inputs["bc"], dtype=np.float32).reshape(-1, 1)
    beta = np.asarray(inputs["beta"], dtype=np.float32)
    beta128 = np.repeat(beta.reshape(1, 1), P, axis=0).astype(np.float32)

    NP, NBP = prep["NP"], prep["NBP"]
    in_maps = []
    for p in range(n_cores):
        xp = x[prep["perm"][p]]
        xpad = np.zeros((NBP, x.shape[1]), np.float32)
        xpad[:NP] = xp
        in_maps.append(
            {
                "xT": np.ascontiguousarray(xpad.T),
                "idx": prep["idx_all"][p],
                "negnpad": prep["negnpad"][p],
                "W1": W1, "b1": b1, "W2": W2, "b2": b2,
                "Wc": Wc, "bc": bc, "beta128": beta128,
            }
        )
    return in_maps


def _postprocess(results, prep, n_cores=CORES):
    N, NP = prep["N"], prep["NP"]
    C = results[0]["out"].shape[0]
    y = np.zeros((N, C), np.float32)
    for p in range(n_cores):
        yp = results[p]["out"].T[:NP]
        y[prep["perm"][p]] = yp
    return y


def kernel(**inputs):
    from concourse.bass_utils import run_bass_kernel_spmd

    x = np.asarray(inputs["x"])
    prep = _preprocess(x, np.asarray(inputs["edge_index"]))
    cfg = dict(prep)
    cfg["HID"] = int(np.asarray(inputs["W2"]).shape[0])
    cfg["C"] = int(np.asarray(inputs["Wc"]).shape[1])
    cfg["n_cores"] = CORES
    cfg_key = (cfg["N"], cfg["IN"], cfg["HID"], cfg["C"],
               tuple(int(k) for k in cfg["K"].ravel()))
    nc = _get_nc(cfg_key, cfg)
    in_maps = _make_in_maps(inputs, prep)
    res = run_bass_kernel_spmd(nc, in_maps, core_ids=list(range(CORES)))
    return _postprocess(res.results, prep)


if __name__ == "__main__":
    pass

